# revision 58
# baseline (speedup 1.0000x reference)
"""Bass/Trainium2 kernel for a 2-layer GAT (nn_GAT_48919677501958).

Contract: kernel(**inputs) takes the FULL unsharded numpy inputs (keyed as in
setup_inputs()) and returns the FULL [10000, 40] float32 output.

Strategy (8 NeuronCores, SPMD single program), slot-major edge phase:
  - Host: append self-loops, degree-sort nodes into 128-node dst tiles,
    snake-assign 10 tiles per core. Edges of each tile are laid out in CSR
    (dst-grouped) "slot" order and padded to 128-slot chunks. Per chunk a 0/1
    segment matrix S [slot, dst] (and its transpose ST) is precomputed on the
    host (fp8 data input).
  - Device per core:
      Phase A: sharded H = X@W1 (+ attention halves) -> DRAM table
               haug [10240, rw1]: row = [h(512, head-interleaved) | a_src(8)],
               AllGather across cores.
      Phase B per tile: one dma_gather of all edge rows (slot i -> partition
               i%128, chunk i//128). Per chunk, PE matmuls with the masks do
               all segment work: a_dst broadcast (ST @ ad), softmax denom
               (S @ ex, PSUM-accumulated), and the weighted message sum
               (S @ (ex*h), PSUM-accumulated). DVE only computes the per-edge
               logits (tiny) and the ex*h product (bf16 2x mode). The softmax
               division is folded into a per-dst post-scale of the PSUM.
      Phase C: transpose y, layer-2 matmul -> h2 table rows [h2|a_src2] bf16,
               AllGather.
      Phase D: same gather indices + masks against the h2 table (256B rows).
  - Host: concat per-core outputs, inverse-permute rows.
"""

from dataclasses import dataclass, field

import numpy as np

import concourse.bass as bass
import concourse.mybir as mybir
import concourse.tile as tile
from concourse.bass_utils import run_bass_kernel_spmd
from concourse.masks import make_identity

F32 = mybir.dt.float32
F32R = mybir.dt.float32r
BF16 = mybir.dt.bfloat16
F8 = mybir.dt.float8e4
I16 = mybir.dt.int16

NEG_SLOPE = 0.2
P = 128


@dataclass
class Cfg:
    n_nodes: int  # 10000
    n_cores: int  # 8
    tpc: int  # dst tiles per core (10)
    d_in: int  # 256
    hid: int  # 64
    heads: int  # 8
    d_out: int  # 40
    tab: str = "bf16"  # layer-1 gather table dtype: bf16 | fp8
    c_prog: list[int] = field(default_factory=list)  # chunks per tile rank
    collective: bool = True  # False: AllGather -> local copy (cost model)
    debug: bool = False  # dump tile-0 intermediates

    @property
    def npc(self):
        return self.tpc * P

    @property
    def npad(self):
        return self.n_cores * self.npc

    @property
    def d_hid(self):
        return self.hid * self.heads  # 512

    @property
    def rw1(self):  # layer-1 table row elems (bytes %256)
        if self.tab == "fp8":
            return 768  # 512 fp8 h + 8 bf16 a_src (16B) + pad
        return 640  # bf16: 512 h + 8 a_src + pad -> 1280B

    @property
    def rw2(self):  # layer-2 table row (bf16): 40 h2 + 1 a_src2 + pad -> 256B
        return 128

    @property
    def cmax(self):
        return max(self.c_prog)

    @property
    def sum_c(self):
        return sum(self.c_prog)


def _wrap_idx(flat: np.ndarray) -> np.ndarray:
    """dma_gather index layout: position i lives at [i % 16, i // 16],
    replicated across the 8 GpSimd-core stripes of 16 partitions each."""
    assert flat.size % 16 == 0
    w = np.ascontiguousarray(flat.reshape(-1, 16).T).astype(np.int16)
    return np.tile(w, (8, 1))


def preprocess(cfg: Cfg, x, edge_index, W1, att_src1, att_dst1, b1, W2,
               att_src2, att_dst2, b2):
    N = cfg.n_nodes
    src = np.concatenate([np.asarray(edge_index[0], np.int64), np.arange(N)])
    dst = np.concatenate([np.asarray(edge_index[1], np.int64), np.arange(N)])
    deg = np.bincount(dst, minlength=N)

    # CSR by dst
    order_e = np.argsort(dst, kind="stable")
    sorted_src = src[order_e]
    starts = np.zeros(N + 1, np.int64)
    np.cumsum(deg, out=starts[1:])

    # degree-sorted node order, padded with -1 dummies
    node_order = np.argsort(-deg, kind="stable")
    padded = np.full(cfg.npad, -1, np.int64)
    padded[:N] = node_order
    tiles = padded.reshape(-1, P)
    tile_load = np.where(tiles >= 0, deg[np.maximum(tiles, 0)], 1).sum(axis=1)

    # snake-assign tiles to cores, per-core sort by load desc
    core_tiles = [[] for _ in range(cfg.n_cores)]
    for r in range(cfg.tpc):
        row = list(range(r * cfg.n_cores, (r + 1) * cfg.n_cores))
        if r % 2:
            row = row[::-1]
        for c in range(cfg.n_cores):
            core_tiles[c].append(row[c])
    for c in range(cfg.n_cores):
        core_tiles[c].sort(key=lambda i: -tile_load[i])

    cfg.c_prog = [
        int(max(-(-tile_load[core_tiles[c][t]] // P) for c in range(cfg.n_cores)))
        for t in range(cfg.tpc)
    ]

    # slot -> node map and node -> table row map
    node_of_slot = np.full((cfg.n_cores, cfg.npc), -1, np.int64)
    for c in range(cfg.n_cores):
        for t in range(cfg.tpc):
            node_of_slot[c, t * P:(t + 1) * P] = tiles[core_tiles[c][t]]
    row_of_node = np.full(N, -1, np.int64)
    flat_slots = node_of_slot.reshape(-1)
    real = flat_slots >= 0
    row_of_node[flat_slots[real]] = np.nonzero(real)[0]
    assert (row_of_node >= 0).all()

    # permuted, padded, transposed x (own columns per core)
    xT = np.zeros((cfg.d_in, cfg.npad), np.float32)
    xT[:, np.nonzero(real)[0]] = np.asarray(x, np.float32).T[:, flat_slots[real]]

    # packed weights (host folding)
    W1 = np.asarray(W1, np.float32)
    ablk_s = np.zeros((cfg.d_hid, cfg.heads), np.float32)
    ablk_d = np.zeros((cfg.d_hid, cfg.heads), np.float32)
    a_s1 = np.asarray(att_src1, np.float32)
    a_d1 = np.asarray(att_dst1, np.float32)
    for h in range(cfg.heads):
        ablk_s[h * cfg.hid:(h + 1) * cfg.hid, h] = a_s1[h]
        ablk_d[h * cfg.hid:(h + 1) * cfg.hid, h] = a_d1[h]
    Wa1 = np.concatenate([W1 @ ablk_s, W1 @ ablk_d], axis=1)  # [d_in, 16]
    W2 = np.asarray(W2, np.float32)
    w2s = W2 @ np.asarray(att_src2, np.float32)[0]
    w2d = W2 @ np.asarray(att_dst2, np.float32)[0]
    W2a = np.concatenate([W2, w2s[:, None], w2d[:, None]], axis=1)  # [512, 42]
    b1r = np.tile(np.asarray(b1, np.float32)[None, :], (P, 1))
    b2r = np.tile(np.asarray(b2, np.float32)[None, :], (P, 1))
    # head-interleaved hidden layout: new col j=(c,h) maps to old col h*hid+c
    j = np.arange(cfg.d_hid)
    old = (j % cfg.heads) * cfg.hid + j // cfg.heads
    b1r = np.ascontiguousarray(b1r[:, old])
    W2a = np.ascontiguousarray(W2a[old, :])

    bf16 = mybir.dt.np(BF16)
    f8 = mybir.dt.np(F8)

    # per-core slot lists + masks
    in_maps = []
    for c in range(cfg.n_cores):
        gi_parts = []
        S_parts, ST_parts = [], []
        for t in range(cfg.tpc):
            c_t = cfg.c_prog[t]
            L_pad = c_t * P
            nodes = node_of_slot[c, t * P:(t + 1) * P]
            srcs_l, dsts_l = [], []
            for d in range(P):
                n = nodes[d]
                if n >= 0:
                    k = int(deg[n])
                    srcs_l.append(row_of_node[sorted_src[starts[n]:starts[n] + k]])
                    dsts_l.append(np.full(k, d, np.int64))
                else:
                    srcs_l.append(np.zeros(1, np.int64))  # fake edge: den>0
                    dsts_l.append(np.full(1, d, np.int64))
            slots_src = np.concatenate(srcs_l)
            slots_dst = np.concatenate(dsts_l)
            L = slots_src.size
            assert L <= L_pad, (L, L_pad)
            pad = L_pad - L
            slots_src = np.concatenate([slots_src, np.zeros(pad, np.int64)])
            slots_dst = np.concatenate([slots_dst, np.full(pad, -1, np.int64)])
            gi_parts.append(slots_src)

            i = np.arange(L_pad)
            pp, jj = i % P, i // P
            valid = slots_dst >= 0
            S = np.zeros((P, c_t, P), np.uint8)   # [slot_p, chunk, dst]
            ST = np.zeros((P, c_t, P), np.uint8)  # [dst, chunk, slot_p]
            S[pp[valid], jj[valid], slots_dst[valid]] = 1
            ST[slots_dst[valid], jj[valid], pp[valid]] = 1
            S_parts.append(S)
            ST_parts.append(ST)
        gi = _wrap_idx(np.concatenate(gi_parts))
        S_all = np.concatenate(S_parts, axis=1).astype(f8)
        ST_all = np.concatenate(ST_parts, axis=1).astype(f8)
        in_maps.append({
            "xTo": np.ascontiguousarray(xT[:, c * cfg.npc:(c + 1) * cfg.npc]),
            "W1": W1, "Wa1": Wa1,
            "b1r": b1r.astype(bf16), "W2a": W2a.astype(bf16),
            "b2r": b2r, "gi": gi, "Sm": S_all, "STm": ST_all,
        })
    return in_maps, node_of_slot


def build_program(cfg: Cfg) -> bass.Bass:
    import concourse.bacc as bacc
    nc = bacc.Bacc("TRN2", target_bir_lowering=False, num_devices=cfg.n_cores)
    DH, HD, DO = cfg.d_hid, cfg.heads, cfg.d_out
    KT = cfg.d_in // P   # k-tiles layer-1 (2)
    K2 = DH // P         # k-tiles layer-2 (4)
    NIDX = P * cfg.sum_c
    TDT = BF16 if cfg.tab == "bf16" else mybir.dt.float8e3

    # ---- DRAM ----
    xTo = nc.dram_tensor("xTo", [cfg.d_in, cfg.npc], F32, kind="ExternalInput")
    W1 = nc.dram_tensor("W1", [cfg.d_in, DH], F32, kind="ExternalInput")
    Wa1 = nc.dram_tensor("Wa1", [cfg.d_in, 2 * HD], F32, kind="ExternalInput")
    b1r = nc.dram_tensor("b1r", [P, DH], BF16, kind="ExternalInput")
    W2a = nc.dram_tensor("W2a", [DH, DO + 2], BF16, kind="ExternalInput")
    b2r = nc.dram_tensor("b2r", [P, DO], F32, kind="ExternalInput")
    gi = nc.dram_tensor("gi", [P, NIDX // 16], I16, kind="ExternalInput")
    Sm = nc.dram_tensor("Sm", [P, cfg.sum_c, P], F8, kind="ExternalInput")
    STm = nc.dram_tensor("STm", [P, cfg.sum_c, P], F8, kind="ExternalInput")
    out = nc.dram_tensor("out", [cfg.npc, DO], F32, kind="ExternalOutput")
    dbg = (nc.dram_tensor("dbg", [4, P, DH], F32, kind="ExternalOutput")
           if cfg.debug else None)

    shared = "Shared" if cfg.collective else "Local"
    haug = nc.dram_tensor("haug", [cfg.npad, cfg.rw1], TDT, addr_space=shared)
    haug_own = nc.dram_tensor("haug_own", [cfg.npc, cfg.rw1], TDT)
    h2own = nc.dram_tensor("h2own", [cfg.npc, cfg.rw2], BF16)
    h2all = nc.dram_tensor("h2all", [cfg.npad, cfg.rw2], BF16,
                           addr_space=shared)

    from contextlib import ExitStack
    with tile.TileContext(nc) as tc, ExitStack() as st:
        cst = st.enter_context(tc.tile_pool(name="cst", bufs=1))
        hsb_p = st.enter_context(tc.tile_pool(name="hsb", bufs=4))
        hg_p = st.enter_context(tc.tile_pool(name="hg", bufs=8))
        hg2_p = st.enter_context(tc.tile_pool(name="hg2", bufs=5))
        sm_p = st.enter_context(tc.tile_pool(name="sm", bufs=8))
        big_p = st.enter_context(tc.tile_pool(name="big", bufs=4))
        out_p = st.enter_context(tc.tile_pool(name="outp", bufs=4))
        psS_p = st.enter_context(tc.tile_pool(name="psS", bufs=2, space="PSUM"))
        psA_p = st.enter_context(tc.tile_pool(name="psA", bufs=2, space="PSUM"))
        psT_p = st.enter_context(tc.tile_pool(name="psT", bufs=2, space="PSUM"))
        psDEN_p = st.enter_context(
            tc.tile_pool(name="psDEN", bufs=2, space="PSUM"))

        # ---- constants ----
        w1sb = cst.tile([P, KT, DH], BF16)
        wa1sb = cst.tile([P, KT, 2 * HD], BF16)
        w2sb = cst.tile([P, K2, DO + 2], BF16)
        b1sb = cst.tile([P, DH], BF16)
        b2sb = cst.tile([P, DO], F32)
        gisb = cst.tile([P, NIDX // 16], I16)
        ident = cst.tile([P, P], BF16)
        ad_sb = cst.tile([P, cfg.tpc * HD], BF16)
        ad2_sb = cst.tile([P, cfg.tpc], BF16)
        xosb = cst.tile([P, KT, cfg.npc], BF16)
        S_sb = cst.tile([P, cfg.sum_c, P], F8)
        ST_sb = cst.tile([P, cfg.sum_c, P], F8)
        for k in range(KT):
            nc.gpsimd.dma_start(out=xosb[:, k, :], in_=xTo[k * P:(k + 1) * P, :])
        for k in range(KT):
            nc.gpsimd.dma_start(out=w1sb[:, k, :], in_=W1[k * P:(k + 1) * P, :])
            nc.gpsimd.dma_start(out=wa1sb[:, k, :],
                                in_=Wa1[k * P:(k + 1) * P, :])
        for k in range(K2):
            nc.sync.dma_start(out=w2sb[:, k, :], in_=W2a[k * P:(k + 1) * P, :])
        nc.sync.dma_start(out=b1sb[:], in_=b1r[:])
        nc.sync.dma_start(out=b2sb[:], in_=b2r[:])
        # index load off the sync queue; masks load per tile inside Phase B
        nc.scalar.dma_start(out=gisb[:], in_=gi[:])
        make_identity(nc, ident[:])

        # ---- Phase A: haug_own rows = [h (ilv) | a_src | pad] ----
        # alternate PSUM pools (B-phase pools idle here) so 4 tiles pipeline
        for t in range(cfg.tpc):
            lt = xosb[:, :, t * P:(t + 1) * P]
            if t % 2 == 0:
                ph = psA_p.tile([P, DH], F32, tag="agg")
                pa_t = psS_p.tile([P, DH], F32, tag="small")
            else:
                ph = psT_p.tile([P, DH], F32, tag="ct")
                pa_t = psDEN_p.tile([P, 2 * HD], F32, tag="den")
            pa = pa_t[:, :2 * HD]
            for k in range(KT):
                nc.tensor.matmul(ph[:], lt[:, k, :], w1sb[:, k, :],
                                 start=(k == 0), stop=(k == KT - 1))
            for k in range(KT):
                nc.tensor.matmul(pa[:], lt[:, k, :], wa1sb[:, k, :],
                                 start=(k == 0), stop=(k == KT - 1))
            hs = hsb_p.tile([P, cfg.rw1], TDT, tag="hsb")
            nc.scalar.copy(
                hs[:, :DH].rearrange("p (c h) -> p h c", h=HD),
                ph[:].rearrange("p (h c) -> p h c", h=HD))
            if cfg.tab == "bf16":
                nc.scalar.copy(hs[:, DH:DH + HD], pa[:, :HD])
                nc.vector.memset(hs[:, DH + HD:], 0.0)
            else:
                asv = hs[:, DH:DH + 2 * HD].bitcast(BF16)
                nc.scalar.copy(asv[:, :HD], pa[:, :HD])
                nc.vector.memset(hs[:, DH + 2 * HD:], 0.0)
            nc.scalar.copy(ad_sb[:, t * HD:(t + 1) * HD], pa[:, HD:2 * HD])
            hdst = haug_own if cfg.collective else haug
            nc.sync.dma_start(out=hdst[t * P:(t + 1) * P, :], in_=hs[:])

        if cfg.collective:
            nc.gpsimd.collective_compute(
                "AllGather", mybir.AluOpType.bypass,
                ins=[haug_own[:]], outs=[haug[:]],
                replica_groups=[list(range(cfg.n_cores))])

        # ---- Phase B + C per tile ----
        GRP = 8  # chunks per pipeline group
        gi_off = 0  # index positions
        c_off = 0   # chunk offset into masks
        for t in range(cfg.tpc):
            c_t = cfg.c_prog[t]
            groups = [(j0, min(j0 + GRP, c_t)) for j0 in range(0, c_t, GRP)]
            St = S_sb[:, c_off:c_off + c_t, :]
            STt = ST_sb[:, c_off:c_off + c_t, :]
            nc.scalar.dma_start(out=St, in_=Sm[:, c_off:c_off + c_t, :])
            nc.scalar.dma_start(out=STt, in_=STm[:, c_off:c_off + c_t, :])
            adv = ad_sb[:, t * HD:(t + 1) * HD]
            ps_small = psS_p.tile([P, DH], F32, tag="small")
            ps_ad = ps_small[:, :cfg.cmax * HD]
            # dedicated bank: the den accumulation group stays open across
            # the whole tile while other matmuls start groups elsewhere
            ps_den_t = psDEN_p.tile([P, 2 * HD], F32, tag="den")
            ps_den = ps_den_t[:, :HD]
            exs = sm_p.tile([P, cfg.cmax, HD], BF16, tag="ex")
            ps_agg = psA_p.tile([P, DH], F32, tag="agg")
            for j0, j1 in groups:
                gc = j1 - j0
                nidx = P * gc
                hgt = hg_p.tile([P, GRP, cfg.rw1], TDT, tag="hg")
                nc.gpsimd.dma_gather(
                    out_ap=hgt[:, :gc, :], in_ap=haug[:, :],
                    idxs_ap=gisb[:, gi_off // 16:(gi_off + nidx) // 16],
                    num_idxs=nidx, num_idxs_reg=nidx, elem_size=cfg.rw1,
                    single_packet=False)
                gi_off += nidx
                if cfg.tab == "bf16":
                    asv = hgt[:, :gc, DH:DH + HD]
                else:
                    asv = hgt[:, :gc, DH:DH + 2 * HD].bitcast(BF16)[:, :, :HD]
                # logits on PE: ps_ad[jc] = ST_jc @ a_dst + I @ a_src[jc]
                for jc in range(j0, j1):
                    pad_j = ps_ad[:, jc * HD:(jc + 1) * HD]
                    nc.tensor.matmul(pad_j, STt[:, jc, :], adv,
                                     start=True, stop=False)
                    nc.tensor.matmul(pad_j, ident[:], asv[:, jc - j0, :],
                                     start=False, stop=True)
                # ex = exp(lrelu(e)): lrelu decomposed on DVE, exp on ACT
                exv = exs[:, j0:j1, :]
                epsv = ps_ad[:, j0 * HD:j1 * HD].rearrange(
                    "p (c h) -> p c h", h=HD)
                # lrelu via ACT relus (same act table as Exp): relu(e) and
                # relu(-e) = -min(e,0), combined on DVE
                neg = sm_p.tile([P, GRP, HD], BF16, tag="neg")
                negv = neg[:, :gc, :]
                nc.scalar.activation(negv, epsv,
                                     mybir.ActivationFunctionType.Relu,
                                     scale=-1.0)
                nc.scalar.activation(exv, epsv,
                                     mybir.ActivationFunctionType.Relu)
                nc.vector.scalar_tensor_tensor(
                    out=exv, in0=negv, scalar=-NEG_SLOPE, in1=exv,
                    op0=mybir.AluOpType.mult, op1=mybir.AluOpType.add)
                nc.scalar.activation(exv, exv,
                                     mybir.ActivationFunctionType.Exp)
                # denom accumulate: S @ ex
                for jc in range(j0, j1):
                    nc.tensor.matmul(ps_den[:], St[:, jc, :], exs[:, jc, :],
                                     start=(jc == 0), stop=(jc == c_t - 1))
                # weighted messages (DVE 2x), then aggregate: S @ (ex*h)
                hgm = hgt[:, :gc, :DH].rearrange("p s (c h) -> p s c h", h=HD)
                nc.vector.tensor_tensor(
                    out=hgm, in0=hgm,
                    in1=exv.unsqueeze(2).broadcast_to([P, gc, cfg.hid, HD]),
                    op=mybir.AluOpType.mult)
                for jc in range(j0, j1):
                    nc.tensor.matmul(ps_agg[:], St[:, jc, :],
                                     hgt[:, jc - j0, :DH],
                                     start=(jc == 0), stop=(jc == c_t - 1))
            rec = sm_p.tile([P, HD], F32, tag="rec")
            nc.vector.reciprocal(rec[:], ps_den[:])
            if cfg.debug and t == 0:
                dtile = big_p.tile([P, DH], F32, tag="dbgt")
                nc.vector.tensor_copy(dtile[:, :cfg.cmax * HD],
                                      ps_ad[:, :cfg.cmax * HD])
                nc.sync.dma_start(out=dbg[0], in_=dtile[:])
                dtile2 = big_p.tile([P, DH], F32, tag="dbgt")
                nc.vector.memset(dtile2[:], 0.0)
                nc.vector.tensor_copy(
                    dtile2[:, :c_t * HD],
                    exs[:, :c_t, :].rearrange("p c h -> p (c h)"))
                nc.sync.dma_start(out=dbg[1], in_=dtile2[:])
                dtile3 = big_p.tile([P, DH], F32, tag="dbgt")
                nc.vector.memset(dtile3[:], 0.0)
                nc.vector.tensor_copy(dtile3[:, :HD], ps_den[:])
                nc.sync.dma_start(out=dbg[2], in_=dtile3[:])
            # y = elu(agg * rec + b1)
            y = big_p.tile([P, DH], BF16, tag="y")
            nc.vector.tensor_tensor(
                out=y[:].rearrange("p (c h) -> p c h", h=HD),
                in0=ps_agg[:].rearrange("p (c h) -> p c h", h=HD),
                in1=rec[:].unsqueeze(1).broadcast_to([P, cfg.hid, HD]),
                op=mybir.AluOpType.mult)
            nc.vector.tensor_tensor(out=y[:], in0=y[:], in1=b1sb[:],
                                    op=mybir.AluOpType.add)
            yn = big_p.tile([P, DH], BF16, tag="yn")
            nc.vector.tensor_scalar_min(out=yn[:], in0=y[:], scalar1=0.0)
            nc.vector.tensor_scalar_max(out=y[:], in0=y[:], scalar1=0.0)
            nc.scalar.activation(yn[:], yn[:], mybir.ActivationFunctionType.Exp)
            nc.vector.scalar_tensor_tensor(
                out=y[:], in0=yn[:], scalar=-1.0, in1=y[:],
                op0=mybir.AluOpType.add, op1=mybir.AluOpType.add)
            if cfg.debug and t == 0:
                dtile4 = big_p.tile([P, DH], F32, tag="dbgt")
                nc.vector.tensor_copy(dtile4[:], y[:])
                nc.sync.dma_start(out=dbg[3], in_=dtile4[:])
            # ---- Phase C ----
            yT = big_p.tile([P, K2, P], BF16, tag="yT")
            ps_ct = psT_p.tile([P, DH], F32, tag="ct")
            for k in range(K2):
                pt = ps_ct[:, (k % 2) * 64:(k % 2) * 64 + 64].bitcast(BF16)
                nc.tensor.transpose(pt, y[:, k * P:(k + 1) * P], ident[:])
                nc.scalar.copy(yT[:, k, :], pt)
            p2 = ps_ct[:, 256:256 + DO + 2]
            for k in range(K2):
                nc.tensor.matmul(p2[:], yT[:, k, :], w2sb[:, k, :],
                                 start=(k == 0), stop=(k == K2 - 1))
            h2sb = out_p.tile([P, cfg.rw2], BF16, tag="h2sb")
            nc.scalar.copy(h2sb[:, :DO + 1], p2[:, :DO + 1])
            nc.vector.memset(h2sb[:, DO + 1:], 0.0)
            nc.scalar.copy(ad2_sb[:, t:t + 1], p2[:, DO + 1:DO + 2])
            h2dst = h2own if cfg.collective else h2all
            nc.sync.dma_start(out=h2dst[t * P:(t + 1) * P, :], in_=h2sb[:])
            c_off += c_t

        if cfg.collective:
            nc.gpsimd.collective_compute(
                "AllGather", mybir.AluOpType.bypass,
                ins=[h2own[:]], outs=[h2all[:]],
                replica_groups=[list(range(cfg.n_cores))])

        # ---- Phase D ----
        gi_off = 0
        c_off = 0
        GRPD = 32
        for t in range(cfg.tpc):
            c_t = cfg.c_prog[t]
            groups = [(j0, min(j0 + GRPD, c_t)) for j0 in range(0, c_t, GRPD)]
            hg2 = hg2_p.tile([P, cfg.cmax, cfg.rw2], BF16, tag="hg2")
            St = S_sb[:, c_off:c_off + c_t, :]
            STt = ST_sb[:, c_off:c_off + c_t, :]
            ps_d = psS_p.tile([P, DH], F32, tag="small")
            ps_ad2 = ps_d[:, :cfg.cmax]
            ps_den2_t = psDEN_p.tile([P, 2 * HD], F32, tag="den")
            ps_den2 = ps_den2_t[:, :1]
            ps_o_t = psA_p.tile([P, DH], F32, tag="agg")
            ps_o = ps_o_t[:, :DO]
            ex2 = sm_p.tile([P, cfg.cmax], BF16, tag="ex2")
            for j0, j1 in groups:
                gc = j1 - j0
                nidx = P * gc
                nc.gpsimd.dma_gather(
                    out_ap=hg2[:, j0:j1, :], in_ap=h2all[:, :],
                    idxs_ap=gisb[:, gi_off // 16:(gi_off + nidx) // 16],
                    num_idxs=nidx, num_idxs_reg=nidx, elem_size=cfg.rw2,
                    single_packet=False)
                gi_off += nidx
                for jc in range(j0, j1):
                    pad_j = ps_ad2[:, jc:jc + 1]
                    nc.tensor.matmul(pad_j, STt[:, jc, :],
                                     ad2_sb[:, t:t + 1], start=True, stop=False)
                    nc.tensor.matmul(pad_j, ident[:],
                                     hg2[:, jc, DO:DO + 1],
                                     start=False, stop=True)
                e2v = ex2[:, j0:j1]
                neg2 = sm_p.tile([P, GRPD], BF16, tag="neg2")
                n2v = neg2[:, :gc]
                nc.scalar.activation(n2v, ps_ad2[:, j0:j1],
                                     mybir.ActivationFunctionType.Relu,
                                     scale=-1.0)
                nc.scalar.activation(e2v, ps_ad2[:, j0:j1],
                                     mybir.ActivationFunctionType.Relu)
                nc.vector.scalar_tensor_tensor(
                    out=e2v, in0=n2v, scalar=-NEG_SLOPE, in1=e2v,
                    op0=mybir.AluOpType.mult, op1=mybir.AluOpType.add)
                nc.scalar.activation(e2v, e2v,
                                     mybir.ActivationFunctionType.Exp)
                for jc in range(j0, j1):
                    nc.tensor.matmul(ps_den2[:], St[:, jc, :],
                                     ex2[:, jc:jc + 1],
                                     start=(jc == 0), stop=(jc == c_t - 1))
                nc.vector.tensor_tensor(
                    out=hg2[:, j0:j1, :DO], in0=hg2[:, j0:j1, :DO],
                    in1=ex2[:, j0:j1].unsqueeze(2).broadcast_to([P, gc, DO]),
                    op=mybir.AluOpType.mult)
                for jc in range(j0, j1):
                    nc.tensor.matmul(ps_o[:], St[:, jc, :], hg2[:, jc, :DO],
                                     start=(jc == 0), stop=(jc == c_t - 1))
            rec2 = sm_p.tile([P, 1], F32, tag="rec2")
            nc.vector.reciprocal(rec2[:], ps_den2[:])
            osb = out_p.tile([P, DO], F32, tag="osb")
            nc.vector.tensor_scalar_mul(out=osb[:], in0=ps_o[:],
                                        scalar1=rec2[:, :1])
            nc.vector.tensor_tensor(out=osb[:], in0=osb[:], in1=b2sb[:],
                                    op=mybir.AluOpType.add)
            nc.sync.dma_start(out=out[t * P:(t + 1) * P, :], in_=osb[:])
            c_off += c_t

    nc.compile()
    return nc


def default_cfg() -> Cfg:
    return Cfg(n_nodes=10000, n_cores=8, tpc=10, d_in=256, hid=64, heads=8,
               d_out=40, tab="bf16")


def run(inputs: dict, cfg: Cfg | None = None, **run_kwargs):
    cfg = cfg or default_cfg()
    in_maps, node_of_slot = preprocess(cfg, **inputs)
    nc = build_program(cfg)
    res = run_bass_kernel_spmd(nc, in_maps, list(range(cfg.n_cores)),
                               **run_kwargs)
    outs = np.concatenate([res.results[c]["out"] for c in range(cfg.n_cores)],
                          axis=0)
    full = np.zeros((cfg.n_nodes, cfg.d_out), np.float32)
    flat = node_of_slot.reshape(-1)
    real = flat >= 0
    full[flat[real]] = outs[real]
    return full, res


def kernel(**inputs) -> np.ndarray:
    out, _ = run(inputs)
    return out
